# revision 1
# baseline (speedup 1.0000x reference)
"""Trainium2 Bass kernel for nn_Actor_35502199669063 (dense_mlp).

Network: x[65536,64] -> Linear(64,100)+LeakyReLU -> 100x(Linear(100,100)+LeakyReLU)
         -> Linear(100,1) -> tanh -> (a+1)/2*9+1

Strategy: pure data parallel over 8 NeuronCores (8192 rows each), parameters
replicated. Activations kept feature-major [features, batch] in SBUF so each
layer is one stationary-weight matmul streaming batch columns; biases folded in
via an appended ones-row (K=101). Matmuls run in float32r (full PE rate for
fp32 storage). LeakyReLU drains PSUM through both ScalarE (native Lrelu) and
VectorE (custom fused max(x*a, x) DVE op); the batch is split into two
independent column regions, one per engine, sized to balance their rates, so
the two elementwise pipelines never gate each other across layers.
"""

import numpy as np
from contextlib import ExitStack, nullcontext

import concourse.bacc as bacc
import concourse.tile as tile
from concourse import mybir
from concourse.bass_utils import run_bass_kernel_spmd

# ---- problem constants (hardcoded; kernel.py must be self-contained) ----
B_TOTAL = 65536
N_OBS = 64
W = 100          # layer width
N_HIDDEN = 100   # number of hidden Ws
N_CORES = 8
B = B_TOTAL // N_CORES       # 8192 samples per core
ALPHA = 0.01
MIN_FREQ, MAX_FREQ = 1.0, 10.0
OUT_SCALE = (MAX_FREQ - MIN_FREQ) / 2.0   # 4.5
OUT_BIAS = (MAX_FREQ + MIN_FREQ) / 2.0    # 5.5

N_TILE = 512          # max fp32 matmul moving-dim / one PSUM bank
# Region A (ScalarE, 1.2GHz) vs region B (VectorE, 0.96GHz): sized so both
# engines finish a layer at the same time. Chunks are PSUM-slot sized pieces.
COLS_A = 4416
COLS_B = B - COLS_A   # 3776
CHUNKS_A = (1024, 1024, 1024, 1024, 320)
CHUNKS_B = (1024, 1024, 1024, 704)

FP32 = mybir.dt.float32
# matmul-feeding tensors are declared float32r: 4-byte fp32 storage that the PE
# streams at full rate (plain fp32 matmul runs at 1/4 rate). numpy side is float32.
MMDT = mybir.dt.float32r


def _register_leaky_op():
    """Register a fused one-instruction DVE leaky-relu: out = max(in0*s0, in0).

    Stock DVE needs two instructions (tensor_scalar mult + tensor_tensor max),
    halving VectorE's effective PSUM-drain rate; this custom op restores 1x.
    """
    from concourse import dve_ops
    from concourse.dve_spec import Spec, Src0, C0, maxx, lower, _has_src1
    from concourse.dve_uop import DveOpSpec

    name = "LRELU_ANT"
    for op in dve_ops.OPS:
        if op.name == name:
            return op
    spec = Spec(
        body=maxx(Src0 * C0, Src0),
        reference=lambda in0, in1, s0, s1, imm2: np.maximum(
            in0.astype(np.float32) * np.float32(s0), in0.astype(np.float32)
        ),
    )
    row = dve_ops._CUSTOM_DVE_ROW_BASE + len(dve_ops.OPS)
    assert row < 0x20
    dve_ops._SUB_OPCODE_FOR_NAME[name] = row
    shas = {}
    for ver in ("v3", "v4"):
        tmp = DveOpSpec(name=name, opcode=row, uops=lower(spec, ver=ver),
                        rd1_en=_has_src1(spec))
        shas[ver] = tmp.sha(ver)
    op = dve_ops.DveOp(name, spec, subdim=False, uops_sha=shas)
    dve_ops.OPS.append(op)
    dve_ops.CUSTOM_DVE_SPECS[name] = spec
    return op


LRELU_DVE = _register_leaky_op()


def _mm_tiles(width):
    """Split a chunk width into matmul moving-dim tiles. Tiles must start on a
    PSUM bank boundary (512 fp32) and, for full-rate fp32r, be >=256 wide; all
    chunk widths here are multiples of 256, so: 512-tiles plus a 256 tail."""
    out = []
    o = 0
    while width - o >= N_TILE:
        out.append((o, N_TILE))
        o += N_TILE
    if width > o:
        out.append((o, width - o))
    return out


def build_nc(repeats=1, loop_repeats=1):
    nc = bacc.Bacc("TRN2", target_bir_lowering=False, debug=False)

    xt_ext = nc.declare_dram_parameter("xt", [N_OBS + 1, B], MMDT, isOutput=False)
    ws_ext = nc.declare_dram_parameter("ws", [W + 1, N_HIDDEN * W], MMDT, isOutput=False)
    wi_ext = nc.declare_dram_parameter("wi", [N_OBS + 1, W], MMDT, isOutput=False)
    wo_ext = nc.declare_dram_parameter("wo", [W + 1, 1], MMDT, isOutput=False)
    out_ext = nc.declare_dram_parameter("out", [1, B], FP32, isOutput=True)

    regions = {"A": (0, COLS_A, CHUNKS_A), "B": (COLS_A, COLS_B, CHUNKS_B)}

    with tile.TileContext(nc) as tc, ExitStack() as ctx:
        cpool = ctx.enter_context(tc.tile_pool(name="cpool", bufs=1))

        # DMA priority order: tiny tensors and the first few layers' weights
        # first, so layers 0-8 are never input-starved; bulk arrives behind.
        wi = cpool.tile([N_OBS + 1, W], MMDT)
        nc.sync.dma_start(wi[:], wi_ext[:])
        wo = cpool.tile([W + 1, 1], MMDT)
        nc.sync.dma_start(wo[:], wo_ext[:])

        # per-region ping-pong activation buffers (feature-major, ones row on top)
        hreg = {}
        for r, (base, cols, _chunks) in regions.items():
            for p in (0, 1):
                t = cpool.tile([W + 1, cols], MMDT, tag=f"h{r}{p}")
                nc.sync.dma_start(
                    t[W : W + 1, :], xt_ext[N_OBS : N_OBS + 1, 0:cols]
                )
                hreg[(r, p)] = t

        # first 8 hidden layers' weights as a small priority piece
        ws = cpool.tile([W + 1, N_HIDDEN * W], MMDT)
        nc.sync.dma_start(ws[:, 0 : 8 * W], ws_ext[:, 0 : 8 * W])

        # x transposed; DMA split to match layer-0 consumption order
        xt = cpool.tile([N_OBS + 1, B], MMDT)
        xt_pieces = [
            (0, 512), (COLS_A, 512),                # first halves of A0/B0
            (512, 512), (COLS_A + 512, 512),
            (1024, 1024), (COLS_A + 1024, 1024),    # A chunk 1, B chunk 1
            (2048, COLS_A - 2048),                  # rest of A
            (COLS_A + 2048, B - COLS_A - 2048),     # rest of B
        ]
        for off, wdt in xt_pieces:
            nc.sync.dma_start(xt[:, off : off + wdt], xt_ext[:, off : off + wdt])

        # remaining hidden weights in 3 column ranges
        rest0 = 8 * W
        WSPLIT = (N_HIDDEN * W - rest0) // 3
        for s in range(3):
            a = rest0 + s * WSPLIT
            b_ = rest0 + (s + 1) * WSPLIT if s < 2 else N_HIDDEN * W
            nc.sync.dma_start(ws[:, a:b_], ws_ext[:, a:b_])

        out_sb = cpool.tile([1, B], FP32)

        loop_cm = (
            tc.For_i(0, loop_repeats, 1, name="timing")
            if loop_repeats > 1 else nullcontext()
        )
        with loop_cm:
         for _rep in range(repeats):
          with (
            tc.tile_pool(name="psA", bufs=2, space="PSUM") as psA,
            tc.tile_pool(name="psB", bufs=2, space="PSUM") as psB,
          ):
            pool_for = {"A": psA, "B": psB}

            def leaky(r, out_ap, z_ap):
                if r == "A":
                    nc.scalar.activation(
                        out_ap, z_ap,
                        mybir.ActivationFunctionType.Lrelu, alpha=ALPHA)
                else:
                    nc.vector._custom_dve(LRELU_DVE, out=out_ap, in0=z_ap, s0=ALPHA)

            # layers 0..N_HIDDEN: l=0 is the input layer (K=65), 1..100 hidden
            for l in range(N_HIDDEN + 1):
                # interleave the two regions' chunks so PE feeds both engines
                sched = []
                for r in ("A", "B"):
                    _, _, chunks = regions[r]
                    off = 0
                    for w_ in chunks:
                        sched.append((r, off, w_))
                        off += w_
                sched.sort(key=lambda t: (t[1], t[0] == 'A'))  # DVE (B) first at equal offsets

                for r, off, width in sched:
                    base, cols, _ = regions[r]
                    if l == 0:
                        src = xt[:, base : base + cols]
                        w_ap = wi[:]
                    else:
                        src = hreg[(r, (l - 1) % 2)]
                        w_ap = ws[:, (l - 1) * W : l * W]
                    dst = hreg[(r, l % 2)]
                    z = pool_for[r].tile([W, 1024], FP32, tag=f"z{r}")
                    for (mo, mw) in _mm_tiles(width):
                        nc.tensor.matmul(
                            z[:, mo : mo + mw],
                            w_ap,
                            src[:, off + mo : off + mo + mw],
                            start=True,
                            stop=True,
                        )
                    leaky(r, dst[0:W, off : off + width], z[:, 0:width])

          # output layer: z = wo.T @ h_last; out = tanh(z)*4.5 + 5.5.
          # Reuses the psA/psB slots (partition 0 only) so each region's final
          # chunks pipeline right behind its last hidden layer.
            p_last = N_HIDDEN % 2
            for r, (base, cols, _chunks) in regions.items():
                h_last = hreg[(r, p_last)]
                off = 0
                while off < cols:
                    width = min(1024, cols - off)
                    zo = pool_for[r].tile([W, 1024], FP32, tag=f"z{r}")
                    for (mo, mw) in _mm_tiles(width):
                        nc.tensor.matmul(
                            zo[0:1, mo : mo + mw],
                            wo[:],
                            h_last[:, off + mo : off + mo + mw],
                            start=True,
                            stop=True,
                        )
                    o0 = base + off
                    nc.scalar.activation(
                        out_sb[0:1, o0 : o0 + width],
                        zo[0:1, 0:width],
                        mybir.ActivationFunctionType.Tanh,
                    )
                    # affine pipelined per chunk on VectorE
                    nc.vector.tensor_scalar(
                        out_sb[0:1, o0 : o0 + width],
                        out_sb[0:1, o0 : o0 + width],
                        OUT_SCALE, OUT_BIAS,
                        mybir.AluOpType.mult, mybir.AluOpType.add,
                    )
                    off += width

        nc.sync.dma_start(out_ext[:], out_sb[:])

    nc.compile()
    return nc


_NC_CACHE = {}


def get_nc(repeats=1, loop_repeats=1):
    key = ("nc", repeats, loop_repeats)
    if key not in _NC_CACHE:
        _NC_CACHE[key] = build_nc(repeats, loop_repeats)
    return _NC_CACHE[key]


def make_in_maps(x, W_in, b_in, Ws, bs, W_out, b_out):
    """Host-side prep: shard/transpose/augment. All fp32."""
    x = np.ascontiguousarray(x, dtype=np.float32)
    ws_host = np.empty((W + 1, N_HIDDEN * W), dtype=np.float32)
    for i in range(N_HIDDEN):
        ws_host[0:W, i * W : (i + 1) * W] = Ws[i]
        ws_host[W, i * W : (i + 1) * W] = bs[i]
    wi_host = np.empty((N_OBS + 1, W), dtype=np.float32)
    wi_host[0:N_OBS] = W_in
    wi_host[N_OBS] = b_in
    wo_host = np.empty((W + 1, 1), dtype=np.float32)
    wo_host[0:W] = np.asarray(W_out, dtype=np.float32).reshape(W, 1)
    wo_host[W] = np.float32(b_out).reshape(1)

    in_maps = []
    for c in range(N_CORES):
        shard = x[c * B : (c + 1) * B]          # [B, 64]
        xt_host = np.empty((N_OBS + 1, B), dtype=np.float32)
        xt_host[0:N_OBS] = shard.T
        xt_host[N_OBS] = 1.0
        in_maps.append(
            {"xt": xt_host, "ws": ws_host, "wi": wi_host, "wo": wo_host}
        )
    return in_maps


def kernel(x, W_in, b_in, Ws, bs, W_out, b_out):
    nc = get_nc()
    in_maps = make_in_maps(x, W_in, b_in, Ws, bs, W_out, b_out)
    res = run_bass_kernel_spmd(nc, in_maps, core_ids=list(range(N_CORES)))
    out = np.empty((B_TOTAL, 1), dtype=np.float32)
    for c in range(N_CORES):
        out[c * B : (c + 1) * B, 0] = res.results[c]["out"].reshape(B)
    return out



# revision 11
# speedup vs baseline: 1.1069x; 1.1069x over previous
"""Trainium2 Bass kernel for nn_Actor_35502199669063 (dense_mlp).

Network: x[65536,64] -> Linear(64,100)+LeakyReLU -> 100x(Linear(100,100)+LeakyReLU)
         -> Linear(100,1) -> tanh -> (a+1)/2*9+1

Strategy: pure data parallel over 8 NeuronCores (8192 rows each), parameters
replicated. Activations kept feature-major [features, batch] in SBUF so each
layer is one stationary-weight matmul streaming batch columns; biases folded in
via an appended ones-row (K=101). Matmuls run in float32r (full PE rate for
fp32 storage). LeakyReLU drains PSUM through both ScalarE (native Lrelu) and
VectorE (custom fused max(x*a, x) DVE op); the batch is split into two
independent column regions, one per engine, sized to balance their rates, so
the two elementwise pipelines never gate each other across layers.
"""

import numpy as np
from contextlib import ExitStack, nullcontext

import concourse.bacc as bacc
import concourse.tile as tile
from concourse import mybir
from concourse.bass_utils import run_bass_kernel_spmd

# ---- problem constants (hardcoded; kernel.py must be self-contained) ----
B_TOTAL = 65536
N_OBS = 64
W = 100          # layer width
N_HIDDEN = 100   # number of hidden Ws
N_CORES = 8
B = B_TOTAL // N_CORES       # 8192 samples per core
ALPHA = 0.01
MIN_FREQ, MAX_FREQ = 1.0, 10.0
OUT_SCALE = (MAX_FREQ - MIN_FREQ) / 2.0   # 4.5
OUT_BIAS = (MAX_FREQ + MIN_FREQ) / 2.0    # 5.5

N_TILE = 512          # max fp32 matmul moving-dim / one PSUM bank
# Region A (ScalarE, 1.2GHz) vs region B (VectorE, 0.96GHz): sized so both
# engines finish a layer at the same time. Chunks are PSUM-slot sized pieces.
# Small chunks lead each layer so the early drains retire quickly: with
# bufs=2 pools, the first chunk of layer l+1 reuses the PSUM slot of chunk
# n-2 of layer l, so late fat drains stall the PE at every layer boundary.
CHUNKS_A = (512, 896, 1024, 1024, 896)   # all widths multiples of 64:
CHUNKS_B = (768, 1024, 1024, 1024)       # fp32r matmul moving-dim ISA rule
COLS_A = sum(CHUNKS_A)  # 4435
COLS_B = sum(CHUNKS_B)  # 3757
assert COLS_A + COLS_B == B

FP32 = mybir.dt.float32
# matmul-feeding tensors are declared float32r: 4-byte fp32 storage that the PE
# streams at full rate (plain fp32 matmul runs at 1/4 rate). numpy side is float32.
MMDT = mybir.dt.float32r


def _register_leaky_op():
    """Register a fused one-instruction DVE leaky-relu: out = max(in0*s0, in0).

    Stock DVE needs two instructions (tensor_scalar mult + tensor_tensor max),
    halving VectorE's effective PSUM-drain rate; this custom op restores 1x.
    """
    from concourse import dve_ops
    from concourse.dve_spec import Spec, Src0, C0, maxx, lower, _has_src1
    from concourse.dve_uop import DveOpSpec

    name = "LRELU_ANT"
    for op in dve_ops.OPS:
        if op.name == name:
            return op
    spec = Spec(
        body=maxx(Src0 * C0, Src0),
        reference=lambda in0, in1, s0, s1, imm2: np.maximum(
            in0.astype(np.float32) * np.float32(s0), in0.astype(np.float32)
        ),
    )
    row = dve_ops._CUSTOM_DVE_ROW_BASE + len(dve_ops.OPS)
    assert row < 0x20
    dve_ops._SUB_OPCODE_FOR_NAME[name] = row
    shas = {}
    for ver in ("v3", "v4"):
        tmp = DveOpSpec(name=name, opcode=row, uops=lower(spec, ver=ver),
                        rd1_en=_has_src1(spec))
        shas[ver] = tmp.sha(ver)
    op = dve_ops.DveOp(name, spec, subdim=False, uops_sha=shas)
    dve_ops.OPS.append(op)
    dve_ops.CUSTOM_DVE_SPECS[name] = spec
    return op


LRELU_DVE = _register_leaky_op()


def _mm_tiles(width):
    """Split a chunk width into matmul moving-dim tiles. Tiles must start on a
    PSUM bank boundary (512 fp32) and, for full-rate fp32r, be >=256 wide; all
    chunk widths here are multiples of 256, so: 512-tiles plus a 256 tail."""
    out = []
    o = 0
    while width - o >= N_TILE:
        out.append((o, N_TILE))
        o += N_TILE
    if width > o:
        out.append((o, width - o))
    return out


def build_nc(repeats=1, loop_repeats=1):
    nc = bacc.Bacc("TRN2", target_bir_lowering=False, debug=False)

    xt_ext = nc.declare_dram_parameter("xt", [N_OBS + 1, B], MMDT, isOutput=False)
    ws_ext = nc.declare_dram_parameter("ws", [W + 1, N_HIDDEN * W], MMDT, isOutput=False)
    wi_ext = nc.declare_dram_parameter("wi", [N_OBS + 1, W], MMDT, isOutput=False)
    wo_ext = nc.declare_dram_parameter("wo", [W + 1, 1], MMDT, isOutput=False)
    out_ext = nc.declare_dram_parameter("out", [1, B], FP32, isOutput=True)

    regions = {"A": (0, COLS_A, CHUNKS_A), "B": (COLS_A, COLS_B, CHUNKS_B)}

    with tile.TileContext(nc) as tc, ExitStack() as ctx:
        cpool = ctx.enter_context(tc.tile_pool(name="cpool", bufs=1))

        # DMA priority order: each dma_start occupies the issue path ~650 ns
        # regardless of size, so order strictly by first-use time: wi and the
        # first xt chunks gate the very first matmuls.
        wi = cpool.tile([N_OBS + 1, W], MMDT)
        nc.sync.dma_start(wi[:], wi_ext[:])

        xt = cpool.tile([N_OBS + 1, B], MMDT)
        xt_pieces = []
        for r, base in (("A", 0), ("B", COLS_A)):
            off = 0
            for w_ in regions[r][2]:
                xt_pieces.append((base + off, w_))
                off += w_
        # interleave A/B pieces by fractional position (same order the layer-0
        # matmuls consume them)
        xt_pieces.sort(
            key=lambda t: ((t[0] % COLS_A if t[0] < COLS_A else t[0] - COLS_A)
                           + t[1] / 2)
            / (COLS_A if t[0] < COLS_A else COLS_B)
        )
        for off, wdt in xt_pieces[:2]:
            nc.sync.dma_start(xt[:, off : off + wdt], xt_ext[:, off : off + wdt])

        # first 8 hidden layers' weights next (consumed from ~layer-0-done on)
        ws = cpool.tile([W + 1, N_HIDDEN * W], MMDT)
        nc.sync.dma_start(ws[:, 0 : 8 * W], ws_ext[:, 0 : 8 * W])

        # per-region ping-pong activation buffers (feature-major, ones row on
        # top); ones rows are first read by layer 1's matmuls
        hreg = {}
        for r, (base, cols, _chunks) in regions.items():
            for p in (0, 1):
                t = cpool.tile([W + 1, cols], MMDT, tag=f"h{r}{p}")
                nc.sync.dma_start(
                    t[W : W + 1, :], xt_ext[N_OBS : N_OBS + 1, 0:cols]
                )
                hreg[(r, p)] = t

        wo = cpool.tile([W + 1, 1], MMDT)
        nc.sync.dma_start(wo[:], wo_ext[:])

        for off, wdt in xt_pieces[2:]:
            nc.sync.dma_start(xt[:, off : off + wdt], xt_ext[:, off : off + wdt])

        # remaining hidden weights in 3 column ranges
        rest0 = 8 * W
        WSPLIT = (N_HIDDEN * W - rest0) // 3
        for s in range(3):
            a = rest0 + s * WSPLIT
            b_ = rest0 + (s + 1) * WSPLIT if s < 2 else N_HIDDEN * W
            nc.sync.dma_start(ws[:, a:b_], ws_ext[:, a:b_])

        out_sb = cpool.tile([1, B], FP32)

        loop_cm = (
            tc.For_i(0, loop_repeats, 1, name="timing")
            if loop_repeats > 1 else nullcontext()
        )
        with loop_cm:
         for _rep in range(repeats):
          with (
            tc.tile_pool(name="psA", bufs=2, space="PSUM") as psA,
            tc.tile_pool(name="psB", bufs=2, space="PSUM") as psB,
          ):
            pool_for = {"A": psA, "B": psB}

            def leaky(r, out_ap, z_ap):
                if r == "A":
                    nc.scalar.activation(
                        out_ap, z_ap,
                        mybir.ActivationFunctionType.Lrelu, alpha=ALPHA)
                else:
                    nc.vector._custom_dve(LRELU_DVE, out=out_ap, in0=z_ap, s0=ALPHA)

            # layers 0..N_HIDDEN: l=0 is the input layer (K=65), 1..100 hidden
            for l in range(N_HIDDEN + 1):
                # interleave the two regions' chunks so PE feeds both engines
                sched = []
                for r in ("A", "B"):
                    _, _, chunks = regions[r]
                    off = 0
                    for w_ in chunks:
                        sched.append((r, off, w_))
                        off += w_
                # interleave by fractional midpoint so both engines' drains
                # start early; A first so ScalarE's first drain issues after
                # a single 512-col matmul
                sched.sort(
                    key=lambda t: (
                        (t[1] + t[2] / 2) / regions[t[0]][1],
                        t[0] != "A",
                    )
                )

                for r, off, width in sched:
                    base, cols, _ = regions[r]
                    if l == 0:
                        src = xt[:, base : base + cols]
                        w_ap = wi[:]
                    else:
                        src = hreg[(r, (l - 1) % 2)]
                        w_ap = ws[:, (l - 1) * W : l * W]
                    dst = hreg[(r, l % 2)]
                    z = pool_for[r].tile([W, 1024], FP32, tag=f"z{r}")
                    for (mo, mw) in _mm_tiles(width):
                        nc.tensor.matmul(
                            z[:, mo : mo + mw],
                            w_ap,
                            src[:, off + mo : off + mo + mw],
                            start=True,
                            stop=True,
                        )
                    leaky(r, dst[0:W, off : off + width], z[:, 0:width])

          # output layer: z = wo.T @ h_last; out = tanh(z)*4.5 + 5.5.
          # Reuses the psA/psB slots (partition 0 only) so each region's final
          # chunks pipeline right behind its last hidden layer.
            p_last = N_HIDDEN % 2
            for r, (base, cols, _chunks) in regions.items():
                h_last = hreg[(r, p_last)]
                off = 0
                while off < cols:
                    width = min(1024, cols - off)
                    zo = pool_for[r].tile([W, 1024], FP32, tag=f"z{r}")
                    for (mo, mw) in _mm_tiles(width):
                        nc.tensor.matmul(
                            zo[0:1, mo : mo + mw],
                            wo[:],
                            h_last[:, off + mo : off + mo + mw],
                            start=True,
                            stop=True,
                        )
                    o0 = base + off
                    nc.scalar.activation(
                        out_sb[0:1, o0 : o0 + width],
                        zo[0:1, 0:width],
                        mybir.ActivationFunctionType.Tanh,
                    )
                    # affine pipelined per chunk on VectorE
                    nc.vector.tensor_scalar(
                        out_sb[0:1, o0 : o0 + width],
                        out_sb[0:1, o0 : o0 + width],
                        OUT_SCALE, OUT_BIAS,
                        mybir.AluOpType.mult, mybir.AluOpType.add,
                    )
                    off += width

        nc.sync.dma_start(out_ext[:], out_sb[:])

    nc.compile()
    return nc


_NC_CACHE = {}


def get_nc(repeats=1, loop_repeats=1):
    key = ("nc", repeats, loop_repeats)
    if key not in _NC_CACHE:
        _NC_CACHE[key] = build_nc(repeats, loop_repeats)
    return _NC_CACHE[key]


def make_in_maps(x, W_in, b_in, Ws, bs, W_out, b_out):
    """Host-side prep: shard/transpose/augment. All fp32."""
    x = np.ascontiguousarray(x, dtype=np.float32)
    ws_host = np.empty((W + 1, N_HIDDEN * W), dtype=np.float32)
    for i in range(N_HIDDEN):
        ws_host[0:W, i * W : (i + 1) * W] = Ws[i]
        ws_host[W, i * W : (i + 1) * W] = bs[i]
    wi_host = np.empty((N_OBS + 1, W), dtype=np.float32)
    wi_host[0:N_OBS] = W_in
    wi_host[N_OBS] = b_in
    wo_host = np.empty((W + 1, 1), dtype=np.float32)
    wo_host[0:W] = np.asarray(W_out, dtype=np.float32).reshape(W, 1)
    wo_host[W] = np.float32(b_out).reshape(1)

    in_maps = []
    for c in range(N_CORES):
        shard = x[c * B : (c + 1) * B]          # [B, 64]
        xt_host = np.empty((N_OBS + 1, B), dtype=np.float32)
        xt_host[0:N_OBS] = shard.T
        xt_host[N_OBS] = 1.0
        in_maps.append(
            {"xt": xt_host, "ws": ws_host, "wi": wi_host, "wo": wo_host}
        )
    return in_maps


def kernel(x, W_in, b_in, Ws, bs, W_out, b_out):
    nc = get_nc()
    in_maps = make_in_maps(x, W_in, b_in, Ws, bs, W_out, b_out)
    res = run_bass_kernel_spmd(nc, in_maps, core_ids=list(range(N_CORES)))
    out = np.empty((B_TOTAL, 1), dtype=np.float32)
    for c in range(N_CORES):
        out[c * B : (c + 1) * B, 0] = res.results[c]["out"].reshape(B)
    return out



# revision 13
# speedup vs baseline: 1.1118x; 1.0045x over previous
"""Trainium2 Bass kernel for nn_Actor_35502199669063 (dense_mlp).

Network: x[65536,64] -> Linear(64,100)+LeakyReLU -> 100x(Linear(100,100)+LeakyReLU)
         -> Linear(100,1) -> tanh -> (a+1)/2*9+1

Strategy: pure data parallel over 8 NeuronCores (8192 rows each), parameters
replicated. Activations kept feature-major [features, batch] in SBUF so each
layer is one stationary-weight matmul streaming batch columns; biases folded in
via an appended ones-row (K=101). Matmuls run in float32r (full PE rate for
fp32 storage). LeakyReLU drains PSUM through both ScalarE (native Lrelu) and
VectorE (custom fused max(x*a, x) DVE op); the batch is split into two
independent column regions, one per engine, sized to balance their rates, so
the two elementwise pipelines never gate each other across layers.
"""

import numpy as np
from contextlib import ExitStack, nullcontext

import concourse.bacc as bacc
import concourse.tile as tile
from concourse import mybir
from concourse.bass_utils import run_bass_kernel_spmd

# ---- problem constants (hardcoded; kernel.py must be self-contained) ----
B_TOTAL = 65536
N_OBS = 64
W = 100          # layer width
N_HIDDEN = 100   # number of hidden Ws
N_CORES = 8
B = B_TOTAL // N_CORES       # 8192 samples per core
ALPHA = 0.01
MIN_FREQ, MAX_FREQ = 1.0, 10.0
OUT_SCALE = (MAX_FREQ - MIN_FREQ) / 2.0   # 4.5
OUT_BIAS = (MAX_FREQ + MIN_FREQ) / 2.0    # 5.5

N_TILE = 512          # max fp32 matmul moving-dim / one PSUM bank
# Region A (ScalarE, 1.2GHz) vs region B (VectorE, 0.96GHz): sized so both
# engines finish a layer at the same time. Chunks are PSUM-slot sized pieces.
# Small chunks lead each layer so the early drains retire quickly: with
# bufs=2 pools, the first chunk of layer l+1 reuses the PSUM slot of chunk
# n-2 of layer l, so late fat drains stall the PE at every layer boundary.
CHUNKS_A = (512, 896, 1024, 1024, 960)   # all widths multiples of 64:
CHUNKS_B = (768, 1024, 1024, 960)        # fp32r matmul moving-dim ISA rule
COLS_A = sum(CHUNKS_A)  # 4435
COLS_B = sum(CHUNKS_B)  # 3757
assert COLS_A + COLS_B == B

FP32 = mybir.dt.float32
# matmul-feeding tensors are declared float32r: 4-byte fp32 storage that the PE
# streams at full rate (plain fp32 matmul runs at 1/4 rate). numpy side is float32.
MMDT = mybir.dt.float32r


def _register_leaky_op():
    """Register a fused one-instruction DVE leaky-relu: out = max(in0*s0, in0).

    Stock DVE needs two instructions (tensor_scalar mult + tensor_tensor max),
    halving VectorE's effective PSUM-drain rate; this custom op restores 1x.
    """
    from concourse import dve_ops
    from concourse.dve_spec import Spec, Src0, C0, maxx, lower, _has_src1
    from concourse.dve_uop import DveOpSpec

    name = "LRELU_ANT"
    for op in dve_ops.OPS:
        if op.name == name:
            return op
    spec = Spec(
        body=maxx(Src0 * C0, Src0),
        reference=lambda in0, in1, s0, s1, imm2: np.maximum(
            in0.astype(np.float32) * np.float32(s0), in0.astype(np.float32)
        ),
    )
    row = dve_ops._CUSTOM_DVE_ROW_BASE + len(dve_ops.OPS)
    assert row < 0x20
    dve_ops._SUB_OPCODE_FOR_NAME[name] = row
    shas = {}
    for ver in ("v3", "v4"):
        tmp = DveOpSpec(name=name, opcode=row, uops=lower(spec, ver=ver),
                        rd1_en=_has_src1(spec))
        shas[ver] = tmp.sha(ver)
    op = dve_ops.DveOp(name, spec, subdim=False, uops_sha=shas)
    dve_ops.OPS.append(op)
    dve_ops.CUSTOM_DVE_SPECS[name] = spec
    return op


LRELU_DVE = _register_leaky_op()


def _mm_tiles(width):
    """Split a chunk width into matmul moving-dim tiles. Tiles must start on a
    PSUM bank boundary (512 fp32) and, for full-rate fp32r, be >=256 wide; all
    chunk widths here are multiples of 256, so: 512-tiles plus a 256 tail."""
    out = []
    o = 0
    while width - o >= N_TILE:
        out.append((o, N_TILE))
        o += N_TILE
    if width > o:
        out.append((o, width - o))
    return out


def build_nc(repeats=1, loop_repeats=1):
    nc = bacc.Bacc("TRN2", target_bir_lowering=False, debug=False)

    xt_ext = nc.declare_dram_parameter("xt", [N_OBS + 1, B], MMDT, isOutput=False)
    ws_ext = nc.declare_dram_parameter("ws", [W + 1, N_HIDDEN * W], MMDT, isOutput=False)
    wi_ext = nc.declare_dram_parameter("wi", [N_OBS + 1, W], MMDT, isOutput=False)
    wo_ext = nc.declare_dram_parameter("wo", [W + 1, 1], MMDT, isOutput=False)
    out_ext = nc.declare_dram_parameter("out", [1, B], FP32, isOutput=True)

    regions = {"A": (0, COLS_A, CHUNKS_A), "B": (COLS_A, COLS_B, CHUNKS_B)}

    with tile.TileContext(nc) as tc, ExitStack() as ctx:
        cpool = ctx.enter_context(tc.tile_pool(name="cpool", bufs=1))

        # DMA priority order: each dma_start occupies the issue path ~650 ns
        # regardless of size, so order strictly by first-use time: wi and the
        # first xt chunks gate the very first matmuls.
        wi = cpool.tile([N_OBS + 1, W], MMDT)
        nc.sync.dma_start(wi[:], wi_ext[:])

        xt = cpool.tile([N_OBS + 1, B], MMDT)
        xt_pieces = []
        for r, base in (("A", 0), ("B", COLS_A)):
            off = 0
            for w_ in regions[r][2]:
                xt_pieces.append((base + off, w_))
                off += w_
        # interleave A/B pieces by fractional position (same order the layer-0
        # matmuls consume them)
        xt_pieces.sort(
            key=lambda t: ((t[0] % COLS_A if t[0] < COLS_A else t[0] - COLS_A)
                           + t[1] / 2)
            / (COLS_A if t[0] < COLS_A else COLS_B)
        )
        # xt pieces alternate between the SP and ACT DMA queues so issue
        # (~650 ns per dma_start regardless of size) parallelizes; the ACT
        # queue is otherwise idle until its first drain at ~4.7 us.
        xt_queues = (nc.sync, nc.scalar)
        for i, (off, wdt) in enumerate(xt_pieces):
            xt_queues[i % 2].dma_start(
                xt[:, off : off + wdt], xt_ext[:, off : off + wdt]
            )

        # first 8 hidden layers' weights (consumed from ~layer-0-done on)
        ws = cpool.tile([W + 1, N_HIDDEN * W], MMDT)
        nc.sync.dma_start(ws[:, 0 : 8 * W], ws_ext[:, 0 : 8 * W])

        # per-region ping-pong activation buffers (feature-major, ones row on
        # top); ones rows are first read by layer 1's matmuls. These and the
        # remaining bulk weights go on the gpsimd (SWDGE) queue: slower issue,
        # but fully parallel to the critical xt stream.
        hreg = {}
        for r, (base, cols, _chunks) in regions.items():
            for p in (0, 1):
                t = cpool.tile([W + 1, cols], MMDT, tag=f"h{r}{p}")
                nc.gpsimd.dma_start(
                    t[W : W + 1, :], xt_ext[N_OBS : N_OBS + 1, 0:cols]
                )
                hreg[(r, p)] = t

        wo = cpool.tile([W + 1, 1], MMDT)
        nc.gpsimd.dma_start(wo[:], wo_ext[:])

        # remaining hidden weights in 3 column ranges
        rest0 = 8 * W
        WSPLIT = (N_HIDDEN * W - rest0) // 3
        for s in range(3):
            a = rest0 + s * WSPLIT
            b_ = rest0 + (s + 1) * WSPLIT if s < 2 else N_HIDDEN * W
            nc.gpsimd.dma_start(ws[:, a:b_], ws_ext[:, a:b_])

        out_sb = cpool.tile([1, B], FP32)

        loop_cm = (
            tc.For_i(0, loop_repeats, 1, name="timing")
            if loop_repeats > 1 else nullcontext()
        )
        with loop_cm:
         for _rep in range(repeats):
          with (
            tc.tile_pool(name="psA", bufs=2, space="PSUM") as psA,
            tc.tile_pool(name="psB", bufs=2, space="PSUM") as psB,
          ):
            pool_for = {"A": psA, "B": psB}

            def leaky(r, out_ap, z_ap):
                if r == "A":
                    nc.scalar.activation(
                        out_ap, z_ap,
                        mybir.ActivationFunctionType.Lrelu, alpha=ALPHA)
                else:
                    nc.vector._custom_dve(LRELU_DVE, out=out_ap, in0=z_ap, s0=ALPHA)

            # layers 0..N_HIDDEN: l=0 is the input layer (K=65), 1..100 hidden
            for l in range(N_HIDDEN + 1):
                # interleave the two regions' chunks so PE feeds both engines
                sched = []
                for r in ("A", "B"):
                    _, _, chunks = regions[r]
                    off = 0
                    for w_ in chunks:
                        sched.append((r, off, w_))
                        off += w_
                # interleave by fractional midpoint so both engines' drains
                # start early; A first so ScalarE's first drain issues after
                # a single 512-col matmul
                sched.sort(
                    key=lambda t: (
                        (t[1] + t[2] / 2) / regions[t[0]][1],
                        t[0] != "A",
                    )
                )

                for r, off, width in sched:
                    base, cols, _ = regions[r]
                    if l == 0:
                        src = xt[:, base : base + cols]
                        w_ap = wi[:]
                    else:
                        src = hreg[(r, (l - 1) % 2)]
                        w_ap = ws[:, (l - 1) * W : l * W]
                    dst = hreg[(r, l % 2)]
                    z = pool_for[r].tile([W, 1024], FP32, tag=f"z{r}")
                    for (mo, mw) in _mm_tiles(width):
                        nc.tensor.matmul(
                            z[:, mo : mo + mw],
                            w_ap,
                            src[:, off + mo : off + mo + mw],
                            start=True,
                            stop=True,
                        )
                    leaky(r, dst[0:W, off : off + width], z[:, 0:width])

          # output layer: z = wo.T @ h_last; out = tanh(z)*4.5 + 5.5.
          # Reuses the psA/psB slots (partition 0 only) so each region's final
          # chunks pipeline right behind its last hidden layer.
            p_last = N_HIDDEN % 2
            for r, (base, cols, _chunks) in regions.items():
                h_last = hreg[(r, p_last)]
                off = 0
                while off < cols:
                    width = min(1024, cols - off)
                    zo = pool_for[r].tile([W, 1024], FP32, tag=f"z{r}")
                    for (mo, mw) in _mm_tiles(width):
                        nc.tensor.matmul(
                            zo[0:1, mo : mo + mw],
                            wo[:],
                            h_last[:, off + mo : off + mo + mw],
                            start=True,
                            stop=True,
                        )
                    o0 = base + off
                    nc.scalar.activation(
                        out_sb[0:1, o0 : o0 + width],
                        zo[0:1, 0:width],
                        mybir.ActivationFunctionType.Tanh,
                    )
                    # affine pipelined per chunk on VectorE
                    nc.vector.tensor_scalar(
                        out_sb[0:1, o0 : o0 + width],
                        out_sb[0:1, o0 : o0 + width],
                        OUT_SCALE, OUT_BIAS,
                        mybir.AluOpType.mult, mybir.AluOpType.add,
                    )
                    off += width

        nc.sync.dma_start(out_ext[:], out_sb[:])

    nc.compile()
    return nc


_NC_CACHE = {}


def get_nc(repeats=1, loop_repeats=1):
    key = ("nc", repeats, loop_repeats)
    if key not in _NC_CACHE:
        _NC_CACHE[key] = build_nc(repeats, loop_repeats)
    return _NC_CACHE[key]


def make_in_maps(x, W_in, b_in, Ws, bs, W_out, b_out):
    """Host-side prep: shard/transpose/augment. All fp32."""
    x = np.ascontiguousarray(x, dtype=np.float32)
    ws_host = np.empty((W + 1, N_HIDDEN * W), dtype=np.float32)
    for i in range(N_HIDDEN):
        ws_host[0:W, i * W : (i + 1) * W] = Ws[i]
        ws_host[W, i * W : (i + 1) * W] = bs[i]
    wi_host = np.empty((N_OBS + 1, W), dtype=np.float32)
    wi_host[0:N_OBS] = W_in
    wi_host[N_OBS] = b_in
    wo_host = np.empty((W + 1, 1), dtype=np.float32)
    wo_host[0:W] = np.asarray(W_out, dtype=np.float32).reshape(W, 1)
    wo_host[W] = np.float32(b_out).reshape(1)

    in_maps = []
    for c in range(N_CORES):
        shard = x[c * B : (c + 1) * B]          # [B, 64]
        xt_host = np.empty((N_OBS + 1, B), dtype=np.float32)
        xt_host[0:N_OBS] = shard.T
        xt_host[N_OBS] = 1.0
        in_maps.append(
            {"xt": xt_host, "ws": ws_host, "wi": wi_host, "wo": wo_host}
        )
    return in_maps


def kernel(x, W_in, b_in, Ws, bs, W_out, b_out):
    nc = get_nc()
    in_maps = make_in_maps(x, W_in, b_in, Ws, bs, W_out, b_out)
    res = run_bass_kernel_spmd(nc, in_maps, core_ids=list(range(N_CORES)))
    out = np.empty((B_TOTAL, 1), dtype=np.float32)
    for c in range(N_CORES):
        out[c * B : (c + 1) * B, 0] = res.results[c]["out"].reshape(B)
    return out

